# revision 31
# baseline (speedup 1.0000x reference)
"""Bass/Tile Trainium2 kernel for nn_Attention3 (additive/Bahdanau attention).

reference:
    q = decoder_hidden @ W_w.T + W_b          # [B, 1, D]
    k = encoder_outputs @ U_w.T + U_b         # [B, L, D]
    scores = tanh(q + k) @ v_w.T + v_b        # [B, L, 1]
    attn = softmax(scores[..., 0])[:, None]   # [B, 1, L]
    context = attn @ encoder_outputs          # [B, 1, D]
    returns (context, attn)

Sharding: data-parallel over batch B=32 across 8 cores (4 batches/core),
weights replicated.  Heavy matmuls in bf16 with fp32 PSUM accumulation.

Device algorithm per core (layouts transposed so D is on partitions):
  kT[dout, l]   = sum_k UT[k, dout] * encT[k, l]        (PE, bf16, k-major waves)
  tanh tiles    = tanh(kT + (q + W_b + U_b))            (ACT, bias per-partition)
  scores[1, l]  = sum_dout vT[dout] * tanh[dout, l]     (PE, v stationary)
  exp row       = exp(scores)  (+ per-chunk Z sums via accum_out)   (ACT)
  exp_bc        = exp row broadcast to 128 partitions   (SWDGE DMA, casts to bf16)
  ctx partials  = sum_l encT[d, l] * exp_bc[d, l]       (DVE tensor_tensor_reduce)
  ctx           = (sum of partials) * 1/Z               (DVE)  [partition-major out]
  attn          = exp * 1/Z
v_b is dropped: softmax is invariant to a constant score offset.
exp without max-subtraction is safe: |scores| <= sum|v_d| = 32 here (~1.5 in
practice); exp(32) is finite in fp32.
"""

import os
import sys

sys.path.insert(0, "/opt/trn_rl_repo")

import numpy as np
import ml_dtypes

from contextlib import ExitStack

import concourse.bass as bass
import concourse.mybir as mybir
import concourse.tile as tile
from concourse import bacc
from concourse.bass_utils import run_bass_kernel_spmd


def _install_ntff_hook_shim():
    """This image's ``antenv`` lacks ``axon_hooks``, so bass_utils' trace=True
    path crashes.  Register an equivalent module backed by the ctypes NTFF
    profiler in trn_agent_boot (libaxon_pjrt.so C ABI)."""
    import types
    import importlib.util

    if importlib.util.find_spec("antenv.axon_hooks") is not None:
        return
    try:
        if "/root/.axon_site" not in sys.path:
            sys.path.insert(0, "/root/.axon_site")
        from trn_agent_boot.trn_boot import _ntff_profile_via_ctypes

        hook = _ntff_profile_via_ctypes("/opt/axon/libaxon_pjrt.so")
    except Exception:
        hook = None
    mod = types.ModuleType("antenv.axon_hooks")
    mod._hook = hook
    mod.get_axon_ntff_profile_hook = lambda: mod._hook
    mod.set_axon_ntff_profile_hook = lambda h: setattr(mod, "_hook", h)
    sys.modules["antenv.axon_hooks"] = mod


_install_ntff_hook_shim()

BF16 = mybir.dt.bfloat16
F32 = mybir.dt.float32
NPBF16 = ml_dtypes.bfloat16

B, L, D = 32, 2048, 1024
NCORES = 8
NB = B // NCORES          # batches per core = 4
P = 128
KT = D // P               # 8 contraction tiles
MT = D // P               # 8 output-dim tiles
NCHUNK = 512              # matmul moving free dim / PSUM bank
CPB = L // NCHUNK         # 4 chunks per batch
LT = L // P               # 16 l-tiles per batch
MWAVE = 4                 # m-tiles per PSUM wave
KMAJOR = os.environ.get("KMAJOR", "1") == "1"
USE_TTR = os.environ.get("USE_TTR", "1") == "1"
ACT_TANH = mybir.ActivationFunctionType.Tanh
ACT_EXP = mybir.ActivationFunctionType.Exp


def build_nc() -> bass.Bass:
    # Bacc (not raw Bass): its compile() splits multi-sem waits into event
    # semaphores (TRN2 allows 1 embedded wait/instruction) and inserts ACT
    # table loads.
    nc = bacc.Bacc()

    encT = nc.dram_tensor("encT", [NB, D, L], BF16, kind="ExternalInput")
    WT = nc.dram_tensor("WT", [D, D], BF16, kind="ExternalInput")
    UT = nc.dram_tensor("UT", [D, D], BF16, kind="ExternalInput")
    decT = nc.dram_tensor("decT", [D, NB], BF16, kind="ExternalInput")
    biasWU = nc.dram_tensor("biasWU", [P, MT], F32, kind="ExternalInput")
    vT = nc.dram_tensor("vT", [P, MT], BF16, kind="ExternalInput")

    ctxpm_out = nc.dram_tensor("ctxpm", [NB, P, KT], F32, kind="ExternalOutput")
    attn_out = nc.dram_tensor("attn", [NB, L], F32, kind="ExternalOutput")
    z_out = nc.dram_tensor("zout", [NB, 1], F32, kind="ExternalOutput")

    with tile.TileContext(nc) as tc, ExitStack() as ctx:
        consts = ctx.enter_context(tc.tile_pool(name="consts", bufs=1))
        io = ctx.enter_context(tc.tile_pool(name="io", bufs=2))
        big = ctx.enter_context(tc.tile_pool(name="big", bufs=3))
        psA = ctx.enter_context(tc.tile_pool(name="psA", bufs=6, space="PSUM"))
        psB = ctx.enter_context(tc.tile_pool(name="psB", bufs=2, space="PSUM"))
        dram = ctx.enter_context(tc.tile_pool(name="dram", bufs=2, space="DRAM"))

        # ---- constants: q-path consts on the ACT HWDGE ring, proj-path on
        # the SP ring, so both streams start immediately and in parallel.
        # Each dma_start occupies its issuing engine ~0.7us, so split tensors
        # in at most two pieces (first k-slice, rest).
        decT_sb = consts.tile([P, KT, NB], BF16, tag="decT")
        nc.scalar.dma_start(decT_sb, decT.rearrange("(ko p) b -> p ko b", p=P))
        WT_sb = consts.tile([P, KT, D], BF16, tag="WT")
        WT_r = WT.rearrange("(ko p) m -> p ko m", p=P)
        nc.scalar.dma_start(WT_sb[:, 0:2, :], WT_r[:, 0:2, :])
        nc.scalar.dma_start(WT_sb[:, 2:, :], WT_r[:, 2:, :])
        bias_sb = consts.tile([P, MT], F32, tag="bias")
        nc.scalar.dma_start(bias_sb, biasWU[:, :])
        vT_sb = consts.tile([P, MT], BF16, tag="vT")
        nc.scalar.dma_start(vT_sb, vT[:, :])

        UT_sb = consts.tile([P, KT, D], BF16, tag="UT")
        UT_r = UT.rearrange("(ko p) m -> p ko m", p=P)
        nc.sync.dma_start(UT_sb[:, 0:2, :], UT_r[:, 0:2, :])
        nc.sync.dma_start(UT_sb[:, 2:, :], UT_r[:, 2:, :])


        # ---- q^T = W @ dec^T, then qb = q^T + (W_b + U_b) ------------------
        # Emitted inside the first chunk (after its first PE wave) so the
        # projection starts as soon as UT/encT arrive, while WT loads.
        qb_sb = consts.tile([P, MT, NB], F32, tag="qb")

        def emit_q():
            for m in range(MT):
                ps_q = psB.tile([P, NCHUNK], F32, tag="row", name="ps_q")
                for k in range(KT):
                    nc.tensor.matmul(
                        ps_q[:, :NB],
                        WT_sb[:, k, m * P : (m + 1) * P],
                        decT_sb[:, k, :],
                        start=(k == 0),
                        stop=(k == KT - 1),
                    )
                # ScalarE Identity-with-bias (TensorScalarPtr on DVE has a
                # 1-sync-wait limit; this op needs two waits).
                nc.scalar.activation(
                    qb_sb[:, m, :],
                    ps_q[:, :NB],
                    mybir.ActivationFunctionType.Identity,
                    bias=bias_sb[:, m : m + 1],
                    scale=1.0,
                )

        # ---- per-batch state ----------------------------------------------
        exp_rows = {}
        zparts = {}
        ctx_parts = {}

        def emit_chunk(b: int, c: int):
            if c == 0:
                exp_rows[b] = io.tile([1, L], F32, tag="exp_row", name="exp_row")
                zparts[b] = io.tile([1, CPB], F32, tag="zpart", name="zpart")
                ctx_parts[b] = io.tile([P, KT, CPB], F32, tag="ctx_part", name="ctx_part")

            et = big.tile([P, KT, NCHUNK], BF16, tag="encT", name="et", bufs=5)
            eview = encT[b].rearrange("(ko p) l -> p ko l", p=P)[
                :, :, c * NCHUNK : (c + 1) * NCHUNK
            ]
            if b == 0 and c == 0:
                # sliced so the first wave's matmuls start as data arrives
                for k in range(KT):
                    nc.sync.dma_start(et[:, k, :], eview[:, k, :])
            else:
                nc.sync.dma_start(et, eview)

            th = big.tile([P, MT, NCHUNK], BF16, tag="tanh", name="th")
            # k-major waves: matmul (k, m) consumes UT slice k + et slice
            # k, so startup streams; 4 PSUM banks per wave.
            for w in range(MT // MWAVE):
                pks = [
                    psA.tile([P, NCHUNK], F32, tag="kproj", name="pk")
                    for _ in range(MWAVE)
                ]
                for k in range(KT):
                    for mi in range(MWAVE):
                        m = w * MWAVE + mi
                        nc.tensor.matmul(
                            pks[mi],
                            UT_sb[:, k, m * P : (m + 1) * P],
                            et[:, k, :],
                            start=(k == 0),
                            stop=(k == KT - 1),
                        )
                if b == 0 and c == 0 and w == 0:
                    emit_q()  # runs on PE while this wave's tanh drains
                for mi in range(MWAVE):
                    m = w * MWAVE + mi
                    nc.scalar.activation(
                        th[:, m, :],
                        pks[mi],
                        ACT_TANH,
                        bias=qb_sb[:, m, b : b + 1],
                        scale=1.0,
                    )

            ss = psB.tile([1, NCHUNK], F32, tag="row", name="ss")
            for m in range(MT):
                nc.tensor.matmul(
                    ss,
                    vT_sb[:, m : m + 1],
                    th[:, m, :],
                    start=(m == 0),
                    stop=(m == MT - 1),
                )
            csl = slice(c * NCHUNK, (c + 1) * NCHUNK)
            nc.scalar.activation(
                exp_rows[b][:, csl],
                ss,
                ACT_EXP,
                bias=0.0,
                scale=1.0,
                accum_out=zparts[b][:, c : c + 1],
            )

            # broadcast exp chunk to all 128 partitions via a DRAM round-trip:
            # partition-step-0 source APs are legal for DRAM, and SWDGE casts
            # fp32->bf16 on the way back in.  No PE involvement.
            exp_dram = dram.tile([1, NCHUNK], F32, tag="exp_dram", name="exp_dram")
            nc.gpsimd.dma_start(exp_dram, exp_rows[b][:, csl])
            exp_bc = big.tile([P, NCHUNK], BF16, tag="exp_bc", name="exp_bc", bufs=3)
            nc.gpsimd.dma_start(exp_bc, exp_dram.to_broadcast((P, NCHUNK)))

            # ctx partials on DVE: ctx_parts[d, k, c] = sum_l encT[d,l]*exp[l]
            # (tensor_tensor_reduce crashes HW on this build; use mul+reduce)
            prod = big.tile([P, KT, NCHUNK], BF16, tag="prod", name="prod", bufs=2)
            nc.vector.tensor_mul(
                prod, et, exp_bc[:, None, :].to_broadcast((P, KT, NCHUNK))
            )
            nc.vector.reduce_sum(
                ctx_parts[b][:, :, c : c + 1], prod, axis=mybir.AxisListType.X
            )

        def emit_finish(b: int):
            z = io.tile([1, 1], F32, tag="z", name="z")
            nc.vector.reduce_sum(z, zparts[b], axis=mybir.AxisListType.X)
            rz = io.tile([1, 1], F32, tag="rz", name="rz")
            nc.vector.reciprocal(rz, z)

            attn_sb = io.tile([1, L], F32, tag="attn_sb", name="attn_sb")
            nc.vector.tensor_mul(attn_sb, exp_rows[b], rz.to_broadcast((1, L)))
            nc.scalar.dma_start(attn_out[b : b + 1, :], attn_sb)

            nc.scalar.dma_start(z_out[b : b + 1, :], z)
            # unnormalized ctx partials; host divides by Z (a [B,D]-scale epilogue)
            ctx_vec = io.tile([P, KT], F32, tag="ctx_vec", name="ctx_vec")
            nc.vector.reduce_sum(ctx_vec, ctx_parts[b], axis=mybir.AxisListType.X)
            nc.scalar.dma_start(ctxpm_out[b], ctx_vec)

        for b in range(NB):
            for c in range(CPB):
                emit_chunk(b, c)
            emit_finish(b)

    nc.finalize()  # Bacc: runs compile() (wait-splitting, reg alloc, ACT tables)
    return nc


def prep_in_maps(decoder_hidden, encoder_outputs, W_w, W_b, U_w, U_b, v_w, v_b):
    """Host-side shard + layout prep (numpy). All FLOPs stay on device."""
    dec = np.asarray(decoder_hidden, dtype=np.float32)
    enc = np.asarray(encoder_outputs, dtype=np.float32)
    W_w = np.asarray(W_w, dtype=np.float32)
    W_b = np.asarray(W_b, dtype=np.float32)
    U_w = np.asarray(U_w, dtype=np.float32)
    U_b = np.asarray(U_b, dtype=np.float32)
    v_w = np.asarray(v_w, dtype=np.float32)

    WT = np.ascontiguousarray(W_w.T).astype(NPBF16)          # [D, D] = W^T
    UT = np.ascontiguousarray(U_w.T).astype(NPBF16)          # [D, D] = U^T
    biasWU = np.ascontiguousarray((W_b + U_b).reshape(MT, P).T).astype(np.float32)
    vTa = np.ascontiguousarray(v_w[0].reshape(MT, P).T).astype(NPBF16)

    in_maps = []
    for i in range(NCORES):
        sl = slice(i * NB, (i + 1) * NB)
        enc_i = enc[sl]                                       # [NB, L, D]
        in_maps.append(
            {
                "encT": np.ascontiguousarray(enc_i.transpose(0, 2, 1)).astype(NPBF16),
                "WT": WT,
                "UT": UT,
                "decT": np.ascontiguousarray(dec[sl, 0, :].T).astype(NPBF16),
                "biasWU": biasWU,
                "vT": vTa,
            }
        )
    return in_maps


_NC_CACHE = None


def _get_nc():
    global _NC_CACHE
    if _NC_CACHE is None:
        _NC_CACHE = build_nc()
    return _NC_CACHE


def run(inputs: dict, trace: bool = False):
    """Returns ((context, attn), BassKernelResults)."""
    nc = _get_nc()
    in_maps = prep_in_maps(**inputs)
    res = run_bass_kernel_spmd(
        nc, in_maps, core_ids=list(range(NCORES)), trace=trace
    )
    context = np.zeros((B, 1, D), dtype=np.float32)
    attn = np.zeros((B, 1, L), dtype=np.float32)
    for i, r in enumerate(res.results):
        # ctxpm[b] is [P, KT] with d = k*P + p  ->  transpose to [KT, P];
        # divide by Z here (device outputs unnormalized partial sums)
        context[i * NB : (i + 1) * NB, 0, :] = (
            r["ctxpm"].transpose(0, 2, 1).reshape(NB, D) / r["zout"]
        )
        attn[i * NB : (i + 1) * NB, 0, :] = r["attn"]
    return (context, attn), res


def kernel(**inputs):
    out, _ = run(inputs, trace=False)
    return out


if __name__ == "__main__":
    rng = np.random.default_rng(0)
    fake = {
        "decoder_hidden": rng.standard_normal((B, 1, D), dtype=np.float32),
        "encoder_outputs": rng.standard_normal((B, L, D), dtype=np.float32),
        "W_w": rng.uniform(-0.03, 0.03, (D, D)).astype(np.float32),
        "W_b": rng.uniform(-0.03, 0.03, (D,)).astype(np.float32),
        "U_w": rng.uniform(-0.03, 0.03, (D, D)).astype(np.float32),
        "U_b": rng.uniform(-0.03, 0.03, (D,)).astype(np.float32),
        "v_w": rng.uniform(-0.03, 0.03, (1, D)).astype(np.float32),
        "v_b": rng.uniform(-0.03, 0.03, (1,)).astype(np.float32),
    }
    (ctx_o, attn_o), _ = run(fake)
    print("ok", ctx_o.shape, attn_o.shape)


# revision 32
# speedup vs baseline: 1.2121x; 1.2121x over previous
"""Bass/Tile Trainium2 kernel for nn_Attention3 (additive/Bahdanau attention).

reference:
    q = decoder_hidden @ W_w.T + W_b          # [B, 1, D]
    k = encoder_outputs @ U_w.T + U_b         # [B, L, D]
    scores = tanh(q + k) @ v_w.T + v_b        # [B, L, 1]
    attn = softmax(scores[..., 0])[:, None]   # [B, 1, L]
    context = attn @ encoder_outputs          # [B, 1, D]
    returns (context, attn)

Sharding: data-parallel over batch B=32 across 8 cores (4 batches/core),
weights replicated.  Heavy matmuls in bf16 with fp32 PSUM accumulation.

Device algorithm per core (layouts transposed so D is on partitions):
  kT[dout, l]   = sum_k UT[k, dout] * encT[k, l]        (PE, bf16, k-major waves)
  tanh tiles    = tanh(kT + (q + W_b + U_b))            (ACT, bias per-partition)
  scores[1, l]  = sum_dout vT[dout] * tanh[dout, l]     (PE, v stationary)
  exp row       = exp(scores)  (+ per-chunk Z sums via accum_out)   (ACT)
  exp_bc        = exp row broadcast to 128 partitions   (SWDGE DMA, casts to bf16)
  ctx partials  = sum_l encT[d, l] * exp_bc[d, l]       (DVE tensor_tensor_reduce)
  ctx           = (sum of partials) * 1/Z               (DVE)  [partition-major out]
  attn          = exp * 1/Z
v_b is dropped: softmax is invariant to a constant score offset.
exp without max-subtraction is safe: |scores| <= sum|v_d| = 32 here (~1.5 in
practice); exp(32) is finite in fp32.
"""

import os
import sys

sys.path.insert(0, "/opt/trn_rl_repo")

import numpy as np
import ml_dtypes

from contextlib import ExitStack

import concourse.bass as bass
import concourse.mybir as mybir
import concourse.tile as tile
from concourse import bacc
from concourse.bass_utils import run_bass_kernel_spmd


def _install_ntff_hook_shim():
    """This image's ``antenv`` lacks ``axon_hooks``, so bass_utils' trace=True
    path crashes.  Register an equivalent module backed by the ctypes NTFF
    profiler in trn_agent_boot (libaxon_pjrt.so C ABI)."""
    import types
    import importlib.util

    if importlib.util.find_spec("antenv.axon_hooks") is not None:
        return
    try:
        if "/root/.axon_site" not in sys.path:
            sys.path.insert(0, "/root/.axon_site")
        from trn_agent_boot.trn_boot import _ntff_profile_via_ctypes

        hook = _ntff_profile_via_ctypes("/opt/axon/libaxon_pjrt.so")
    except Exception:
        hook = None
    mod = types.ModuleType("antenv.axon_hooks")
    mod._hook = hook
    mod.get_axon_ntff_profile_hook = lambda: mod._hook
    mod.set_axon_ntff_profile_hook = lambda h: setattr(mod, "_hook", h)
    sys.modules["antenv.axon_hooks"] = mod


_install_ntff_hook_shim()

BF16 = mybir.dt.bfloat16
F32 = mybir.dt.float32
NPBF16 = ml_dtypes.bfloat16

B, L, D = 32, 2048, 1024
NCORES = 8
NB = B // NCORES          # batches per core = 4
P = 128
KT = D // P               # 8 contraction tiles
MT = D // P               # 8 output-dim tiles
NCHUNK = 512              # matmul moving free dim / PSUM bank
CPB = L // NCHUNK         # 4 chunks per batch
LT = L // P               # 16 l-tiles per batch
MWAVE = 4                 # m-tiles per PSUM wave
KMAJOR = os.environ.get("KMAJOR", "1") == "1"
USE_TTR = os.environ.get("USE_TTR", "1") == "1"
ACT_TANH = mybir.ActivationFunctionType.Tanh
ACT_EXP = mybir.ActivationFunctionType.Exp


def build_nc() -> bass.Bass:
    # Bacc (not raw Bass): its compile() splits multi-sem waits into event
    # semaphores (TRN2 allows 1 embedded wait/instruction) and inserts ACT
    # table loads.
    nc = bacc.Bacc()

    encT = nc.dram_tensor("encT", [NB, D, L], BF16, kind="ExternalInput")
    WT = nc.dram_tensor("WT", [D, D], BF16, kind="ExternalInput")
    UT = nc.dram_tensor("UT", [D, D], BF16, kind="ExternalInput")
    decT = nc.dram_tensor("decT", [D, NB], BF16, kind="ExternalInput")
    biasWU = nc.dram_tensor("biasWU", [P, MT], F32, kind="ExternalInput")
    vT = nc.dram_tensor("vT", [P, MT], BF16, kind="ExternalInput")

    ctxpm_out = nc.dram_tensor("ctxpm", [NB, P, KT], F32, kind="ExternalOutput")
    attn_out = nc.dram_tensor("attn", [NB, L], F32, kind="ExternalOutput")
    z_out = nc.dram_tensor("zout", [NB, 1], F32, kind="ExternalOutput")

    with tile.TileContext(nc) as tc, ExitStack() as ctx:
        consts = ctx.enter_context(tc.tile_pool(name="consts", bufs=1))
        io = ctx.enter_context(tc.tile_pool(name="io", bufs=2))
        big = ctx.enter_context(tc.tile_pool(name="big", bufs=3))
        psA = ctx.enter_context(tc.tile_pool(name="psA", bufs=6, space="PSUM"))
        psB = ctx.enter_context(tc.tile_pool(name="psB", bufs=2, space="PSUM"))
        dram = ctx.enter_context(tc.tile_pool(name="dram", bufs=2, space="DRAM"))

        # ---- constants: q-path consts on the ACT HWDGE ring, proj-path on
        # the SP ring, so both streams start immediately and in parallel.
        # Each dma_start occupies its issuing engine ~0.7us, so split tensors
        # in at most two pieces (first k-slice, rest).
        decT_sb = consts.tile([P, KT, NB], BF16, tag="decT")
        nc.scalar.dma_start(decT_sb, decT.rearrange("(ko p) b -> p ko b", p=P))
        WT_sb = consts.tile([P, KT, D], BF16, tag="WT")
        WT_r = WT.rearrange("(ko p) m -> p ko m", p=P)
        nc.scalar.dma_start(WT_sb[:, 0:2, :], WT_r[:, 0:2, :])
        nc.scalar.dma_start(WT_sb[:, 2:, :], WT_r[:, 2:, :])
        bias_sb = consts.tile([P, MT], F32, tag="bias")
        nc.scalar.dma_start(bias_sb, biasWU[:, :])
        vT_sb = consts.tile([P, MT], BF16, tag="vT")
        nc.scalar.dma_start(vT_sb, vT[:, :])

        UT_sb = consts.tile([P, KT, D], BF16, tag="UT")
        UT_r = UT.rearrange("(ko p) m -> p ko m", p=P)
        nc.sync.dma_start(UT_sb[:, 0:2, :], UT_r[:, 0:2, :])
        nc.sync.dma_start(UT_sb[:, 2:, :], UT_r[:, 2:, :])


        # ---- q^T = W @ dec^T, then qb = q^T + (W_b + U_b) ------------------
        # Emitted inside the first chunk (after its first PE wave) so the
        # projection starts as soon as UT/encT arrive, while WT loads.
        qb_sb = consts.tile([P, MT, NB], F32, tag="qb")

        def emit_q():
            for m in range(MT):
                ps_q = psB.tile([P, NCHUNK], F32, tag="row", name="ps_q")
                for k in range(KT):
                    nc.tensor.matmul(
                        ps_q[:, :NB],
                        WT_sb[:, k, m * P : (m + 1) * P],
                        decT_sb[:, k, :],
                        start=(k == 0),
                        stop=(k == KT - 1),
                    )
                # ScalarE Identity-with-bias (TensorScalarPtr on DVE has a
                # 1-sync-wait limit; this op needs two waits).
                nc.scalar.activation(
                    qb_sb[:, m, :],
                    ps_q[:, :NB],
                    mybir.ActivationFunctionType.Identity,
                    bias=bias_sb[:, m : m + 1],
                    scale=1.0,
                )

        # ---- per-batch state ----------------------------------------------
        exp_rows = {}
        zparts = {}
        ctx_parts = {}

        def emit_chunk(b: int, c: int):
            if c == 0:
                exp_rows[b] = io.tile([1, L], F32, tag="exp_row", name="exp_row")
                zparts[b] = io.tile([1, CPB], F32, tag="zpart", name="zpart")
                ctx_parts[b] = io.tile([P, KT, CPB], F32, tag="ctx_part", name="ctx_part")

            et = big.tile([P, KT, NCHUNK], BF16, tag="encT", name="et", bufs=5)
            eview = encT[b].rearrange("(ko p) l -> p ko l", p=P)[
                :, :, c * NCHUNK : (c + 1) * NCHUNK
            ]
            if b == 0 and c == 0:
                # sliced so the first wave's matmuls start as data arrives
                for k in range(KT):
                    nc.sync.dma_start(et[:, k, :], eview[:, k, :])
            else:
                nc.sync.dma_start(et, eview)

            th = big.tile([P, MT, NCHUNK], BF16, tag="tanh", name="th")
            # k-major waves: matmul (k, m) consumes UT slice k + et slice
            # k, so startup streams; 4 PSUM banks per wave.
            for w in range(MT // MWAVE):
                pks = [
                    psA.tile([P, NCHUNK], F32, tag="kproj", name="pk")
                    for _ in range(MWAVE)
                ]
                for k in range(KT):
                    for mi in range(MWAVE):
                        m = w * MWAVE + mi
                        nc.tensor.matmul(
                            pks[mi],
                            UT_sb[:, k, m * P : (m + 1) * P],
                            et[:, k, :],
                            start=(k == 0),
                            stop=(k == KT - 1),
                        )
                if b == 0 and c == 0 and w == 0:
                    emit_q()  # runs on PE while this wave's tanh drains
                for mi in range(MWAVE):
                    m = w * MWAVE + mi
                    nc.scalar.activation(
                        th[:, m, :],
                        pks[mi],
                        ACT_TANH,
                        bias=qb_sb[:, m, b : b + 1],
                        scale=1.0,
                    )

            ss = psB.tile([1, NCHUNK], F32, tag="row", name="ss")
            for m in range(MT):
                nc.tensor.matmul(
                    ss,
                    vT_sb[:, m : m + 1],
                    th[:, m, :],
                    start=(m == 0),
                    stop=(m == MT - 1),
                )
            csl = slice(c * NCHUNK, (c + 1) * NCHUNK)
            nc.scalar.activation(
                exp_rows[b][:, csl],
                ss,
                ACT_EXP,
                bias=0.0,
                scale=1.0,
                accum_out=zparts[b][:, c : c + 1],
            )

            # broadcast exp chunk to all 128 partitions via a DRAM round-trip:
            # partition-step-0 source APs are legal for DRAM, and SWDGE casts
            # fp32->bf16 on the way back in.  No PE involvement.
            exp_dram = dram.tile([1, NCHUNK], F32, tag="exp_dram", name="exp_dram")
            nc.gpsimd.dma_start(exp_dram, exp_rows[b][:, csl])
            exp_bc = big.tile([P, NCHUNK], BF16, tag="exp_bc", name="exp_bc", bufs=3)
            nc.gpsimd.dma_start(exp_bc, exp_dram.to_broadcast((P, NCHUNK)))

            # ctx partials on DVE: ctx_parts[d, k, c] = sum_l encT[d,l]*exp[l]
            # (tensor_tensor_reduce crashes HW on this build; use mul+reduce)
            if os.environ.get("NOCTX", "0") != "1":
                prod = big.tile([P, KT, NCHUNK], BF16, tag="prod", name="prod", bufs=2)
                nc.vector.tensor_mul(
                    prod, et, exp_bc[:, None, :].to_broadcast((P, KT, NCHUNK))
                )
                nc.vector.reduce_sum(
                    ctx_parts[b][:, :, c : c + 1], prod, axis=mybir.AxisListType.X
                )
            else:
                nc.vector.memset(ctx_parts[b][:, :, c : c + 1], 0.0)

        def emit_finish(b: int):
            z = io.tile([1, 1], F32, tag="z", name="z")
            nc.vector.reduce_sum(z, zparts[b], axis=mybir.AxisListType.X)
            rz = io.tile([1, 1], F32, tag="rz", name="rz")
            nc.vector.reciprocal(rz, z)

            attn_sb = io.tile([1, L], F32, tag="attn_sb", name="attn_sb")
            nc.vector.tensor_mul(attn_sb, exp_rows[b], rz.to_broadcast((1, L)))
            nc.scalar.dma_start(attn_out[b : b + 1, :], attn_sb)

            nc.scalar.dma_start(z_out[b : b + 1, :], z)
            # unnormalized ctx partials; host divides by Z (a [B,D]-scale epilogue)
            ctx_vec = io.tile([P, KT], F32, tag="ctx_vec", name="ctx_vec")
            nc.vector.reduce_sum(ctx_vec, ctx_parts[b], axis=mybir.AxisListType.X)
            nc.scalar.dma_start(ctxpm_out[b], ctx_vec)

        for b in range(NB):
            for c in range(CPB):
                emit_chunk(b, c)
            emit_finish(b)

    nc.finalize()  # Bacc: runs compile() (wait-splitting, reg alloc, ACT tables)
    return nc


def prep_in_maps(decoder_hidden, encoder_outputs, W_w, W_b, U_w, U_b, v_w, v_b):
    """Host-side shard + layout prep (numpy). All FLOPs stay on device."""
    dec = np.asarray(decoder_hidden, dtype=np.float32)
    enc = np.asarray(encoder_outputs, dtype=np.float32)
    W_w = np.asarray(W_w, dtype=np.float32)
    W_b = np.asarray(W_b, dtype=np.float32)
    U_w = np.asarray(U_w, dtype=np.float32)
    U_b = np.asarray(U_b, dtype=np.float32)
    v_w = np.asarray(v_w, dtype=np.float32)

    WT = np.ascontiguousarray(W_w.T).astype(NPBF16)          # [D, D] = W^T
    UT = np.ascontiguousarray(U_w.T).astype(NPBF16)          # [D, D] = U^T
    biasWU = np.ascontiguousarray((W_b + U_b).reshape(MT, P).T).astype(np.float32)
    vTa = np.ascontiguousarray(v_w[0].reshape(MT, P).T).astype(NPBF16)

    in_maps = []
    for i in range(NCORES):
        sl = slice(i * NB, (i + 1) * NB)
        enc_i = enc[sl]                                       # [NB, L, D]
        in_maps.append(
            {
                "encT": np.ascontiguousarray(enc_i.transpose(0, 2, 1)).astype(NPBF16),
                "WT": WT,
                "UT": UT,
                "decT": np.ascontiguousarray(dec[sl, 0, :].T).astype(NPBF16),
                "biasWU": biasWU,
                "vT": vTa,
            }
        )
    return in_maps


_NC_CACHE = None


def _get_nc():
    global _NC_CACHE
    if _NC_CACHE is None:
        _NC_CACHE = build_nc()
    return _NC_CACHE


def run(inputs: dict, trace: bool = False):
    """Returns ((context, attn), BassKernelResults)."""
    nc = _get_nc()
    in_maps = prep_in_maps(**inputs)
    res = run_bass_kernel_spmd(
        nc, in_maps, core_ids=list(range(NCORES)), trace=trace
    )
    context = np.zeros((B, 1, D), dtype=np.float32)
    attn = np.zeros((B, 1, L), dtype=np.float32)
    for i, r in enumerate(res.results):
        # ctxpm[b] is [P, KT] with d = k*P + p  ->  transpose to [KT, P];
        # divide by Z here (device outputs unnormalized partial sums)
        context[i * NB : (i + 1) * NB, 0, :] = (
            r["ctxpm"].transpose(0, 2, 1).reshape(NB, D) / r["zout"]
        )
        attn[i * NB : (i + 1) * NB, 0, :] = r["attn"]
    return (context, attn), res


def kernel(**inputs):
    out, _ = run(inputs, trace=False)
    return out


if __name__ == "__main__":
    rng = np.random.default_rng(0)
    fake = {
        "decoder_hidden": rng.standard_normal((B, 1, D), dtype=np.float32),
        "encoder_outputs": rng.standard_normal((B, L, D), dtype=np.float32),
        "W_w": rng.uniform(-0.03, 0.03, (D, D)).astype(np.float32),
        "W_b": rng.uniform(-0.03, 0.03, (D,)).astype(np.float32),
        "U_w": rng.uniform(-0.03, 0.03, (D, D)).astype(np.float32),
        "U_b": rng.uniform(-0.03, 0.03, (D,)).astype(np.float32),
        "v_w": rng.uniform(-0.03, 0.03, (1, D)).astype(np.float32),
        "v_b": rng.uniform(-0.03, 0.03, (1,)).astype(np.float32),
    }
    (ctx_o, attn_o), _ = run(fake)
    print("ok", ctx_o.shape, attn_o.shape)
